# revision 16
# baseline (speedup 1.0000x reference)
"""Trainium2 Bass kernel for the RNODE (ODE-RNN) + per-step MLP model.

Model (reference):
    x_{t+1} = x_t + dt*tanh(A x_t + B u_t + bA)       (sequential, S=262144 steps)
    mem[:, t]  = x_{t+1}
    out = W2 @ relu(W1 @ mem + b1) + b2
    returns (out, mem), both [3, S] fp32.

Parallel-in-time strategy (8 cores, data-parallel over the time axis):
  Work in scaled units y = x/dt so the recurrence is y_{t+1} = y_t + tanh(z_t),
  z_t = (dt*A) y_t + c_t, c = B u + bA.  Each core owns a 32768-step segment
  laid out as 128 chunks (partitions) x 256 steps (free dim).

  Newton/Picard iteration per core ("sweep"):
    1. z = dtA@y + c (fused scalar_tensor_tensor chains), th = tanh(z) (ACT)
    2. per-chunk inclusive prefix sums of th via tensor_tensor_scan
    3. chunk-boundary correction: solve the linearized boundary recurrence
       delta_{c+1} = (I + diag(SD_c) A) delta_c + r_c across the 128 chunks
       (SD_c = dt * sum sech^2, first-order chunk Jacobian) by inner Picard
       iterations using a [3,128]-layout scan + a tiny PE matmul for A@delta.
    4. rebuild trajectory y from corrected boundaries + prefix sums.

  Cross-core coupling is resolved with two launches: launch A runs sweeps from
  a cheap coarse seed; the host composes the 8 per-core first-order boundary
  maps (8 tiny 3x3 affine maps) into per-core incoming-state corrections eps;
  launch B applies eps (delta-scan initial value), re-sweeps, then evaluates
  the MLP on the tensor engine (W1/W2 matmuls, relu+bias fused in the
  PSUM->SBUF copies on ACT/DVE) and streams both outputs to DRAM.

The dynamics saturate tanh (|x| grows to ~84), so sech^2 ~ 0 over most of the
sequence and the first-order chunk Jacobians are nearly exact; the iteration
converges to well inside the fp32 envelope of the sequential reference
(validated against an fp64 sequential solve).
"""

import numpy as np

import concourse.bass as bass
import concourse.bacc as bacc
import concourse.mybir as mybir
from concourse.bass_utils import run_bass_kernel_spmd
from concourse.masks import make_identity
from concourse.tile import TileContext

F32 = mybir.dt.float32
BF16 = mybir.dt.bfloat16
F16 = mybir.dt.float16
AOP = mybir.AluOpType
AFT = mybir.ActivationFunctionType

S = 262144
NCORE = 8
SEG = S // NCORE          # 32768 steps per core
L = 256                   # chunk length (free dim)
CPC = SEG // L            # 128 chunks per core (partition dim)
NBLK = 8                  # MLP time blocks per core
BLK = SEG // NBLK         # 4096 steps per MLP block
CHB = BLK // L            # 16 chunks per MLP block

_CACHE = {}
LAST_TIMES = {}   # filled with exec_time_ns per launch when tracing is on


# --------------------------------------------------------------------------
# shared sweep emitter
# --------------------------------------------------------------------------
class T:  # tile namespace
    pass


def _alloc_common(nc, pool, ppool):
    t = T()
    t.ident = pool.tile([128, 128], F32)
    make_identity(nc, t.ident[:, :])
    t.zeros = pool.tile([128, 3 * L], F32)
    nc.vector.memset(t.zeros[:, :], 0.0)
    t.zeros3 = pool.tile([3, CPC + 4], F32)
    nc.vector.memset(t.zeros3[:, :], 0.0)
    t.ones3 = pool.tile([3, CPC + 4], F32)
    nc.vector.memset(t.ones3[:, :], 1.0)
    t.c = pool.tile([128, 3 * L], F32)
    t.y = pool.tile([128, 3 * L], F32)
    t.z = pool.tile([128, 3 * L], F32)
    t.th = pool.tile([128, 3 * L], F32)
    t.incl = pool.tile([128, 3 * L], F32)
    t.junk = pool.tile([128, L], F32)
    t.small = pool.tile([128, 8], F32)
    t.b = pool.tile([3, CPC + 4], F32)
    t.delta = pool.tile([3, CPC + 4], F32)
    t.r = pool.tile([3, CPC], F32)
    t.g = pool.tile([3, CPC], F32)
    t.SDp = pool.tile([3, CPC], F32)
    t.bcols = pool.tile([128, 3], F32)
    t.at = pool.tile([3, 4], F32)
    t.ptT = ppool.tile([3, 128], F32)
    t.ptS = ppool.tile([3, 128], F32)
    t.pb = ppool.tile([128, 3], F32)
    t.psA = ppool.tile([3, CPC], F32)
    return t


def _emit_sweep(nc, t, k_in, dtA, dtf, eps_ap=None):
    """One outer iteration: z/tanh/prefix + chunk-boundary solve + rebuild."""
    # z_i = sum_k dtA[i,k]*y_k + c_i
    for i in range(3):
        zi = t.z[:, i * L:(i + 1) * L]
        nc.vector.scalar_tensor_tensor(
            zi, t.y[:, 0:L], dtA[i][0], t.c[:, i * L:(i + 1) * L],
            op0=AOP.mult, op1=AOP.add)
        nc.vector.scalar_tensor_tensor(
            zi, t.y[:, L:2 * L], dtA[i][1], zi, op0=AOP.mult, op1=AOP.add)
        nc.vector.scalar_tensor_tensor(
            zi, t.y[:, 2 * L:3 * L], dtA[i][2], zi, op0=AOP.mult, op1=AOP.add)
    nc.scalar.activation(t.th[:, :], t.z[:, :], AFT.Tanh)
    for i in range(3):
        thi = t.th[:, i * L:(i + 1) * L]
        # junk = th^2, accum_out = sum(th^2) per chunk  (for SD)
        nc.vector.scalar_tensor_tensor(
            t.junk[:, :], thi, 1.0, thi, op0=AOP.mult, op1=AOP.mult,
            accum_out=t.small[:, 3 + i:4 + i])
        # inclusive prefix sum of th along the chunk
        nc.vector.tensor_tensor_scan(
            t.incl[:, i * L:(i + 1) * L], t.zeros[:, 0:L], thi, 0.0,
            op0=AOP.add, op1=AOP.add)
        nc.vector.tensor_copy(
            t.small[:, i:i + 1], t.incl[:, (i + 1) * L - 1:(i + 1) * L])
    # chunk summaries -> [*,128] layout
    nc.tensor.transpose(t.ptT[:, :], t.small[:, 0:3], t.ident[:, :])
    nc.tensor.transpose(t.ptS[:, :], t.small[:, 3:6], t.ident[:, :])
    # SD' = dt*(L - sum th^2) = (sth * -dt) + L*dt
    nc.vector.tensor_scalar(
        t.SDp[:, :], t.ptS[:, :], -dtf, float(L) * dtf,
        op0=AOP.mult, op1=AOP.add)
    # defect r_c = b_c + T_c - b_{c+1}
    nc.vector.tensor_tensor(t.r[:, :], t.b[:, 0:CPC], t.ptT[:, :], AOP.add)
    nc.vector.tensor_tensor(t.r[:, :], t.r[:, :], t.b[:, 1:CPC + 1], AOP.subtract)
    # delta init
    if eps_ap is not None:
        nc.vector.tensor_scalar(
            t.delta[:, 0:CPC + 1], t.ones3[:, 0:CPC + 1], eps_ap, None,
            op0=AOP.mult)
    else:
        nc.vector.memset(t.delta[:, 0:CPC + 1], 0.0)
    # inner Picard on the boundary recurrence
    for _ in range(k_in):
        nc.tensor.matmul(t.psA[:, :], t.at[0:3, 0:3], t.delta[:, 0:CPC],
                         start=True, stop=True)
        nc.vector.tensor_tensor(t.g[:, :], t.SDp[:, :], t.psA[:, :], AOP.mult)
        nc.vector.tensor_tensor(t.g[:, :], t.g[:, :], t.r[:, :], AOP.add)
        nc.vector.tensor_tensor_scan(
            t.delta[:, 1:CPC + 1], t.zeros3[:, 0:CPC], t.g[:, :],
            t.delta[:, 0:1], op0=AOP.add, op1=AOP.add)
    nc.vector.tensor_tensor(t.b[:, 0:CPC + 1], t.b[:, 0:CPC + 1],
                            t.delta[:, 0:CPC + 1], AOP.add)
    # refresh per-chunk boundary columns and rebuild y
    nc.tensor.transpose(t.pb[:, :], t.b[:, 0:CPC], t.ident[0:3, 0:3])
    nc.vector.tensor_copy(t.bcols[:, :], t.pb[:, :])
    for i in range(3):
        nc.vector.tensor_copy(t.y[:, i * L:i * L + 1], t.bcols[:, i:i + 1])
        nc.vector.tensor_scalar(
            t.y[:, i * L + 1:(i + 1) * L], t.incl[:, i * L:(i + 1) * L - 1],
            t.bcols[:, i:i + 1], None, op0=AOP.add)


# --------------------------------------------------------------------------
# launch A
# --------------------------------------------------------------------------
def build_A(dtA, Bm, bAv, dtf, k_list):
    nc = bacc.Bacc("TRN2")
    useg = nc.dram_tensor("useg", [3, SEG], F32, kind="ExternalInput")
    bseed = nc.dram_tensor("bseed", [3, CPC + 4], F32, kind="ExternalInput")
    atin = nc.dram_tensor("atin", [3, 4], F32, kind="ExternalInput")
    y_out = nc.dram_tensor("y_out", [128, 3 * L], F32, kind="ExternalOutput")
    c_out = nc.dram_tensor("c_out", [128, 3 * L], F32, kind="ExternalOutput")
    sm_out = nc.dram_tensor("sm_out", [128, 8], F32, kind="ExternalOutput")
    b_out = nc.dram_tensor("b_out", [3, CPC + 4], F32, kind="ExternalOutput")

    with TileContext(nc) as tc:
        with tc.tile_pool(name="p", bufs=1) as pool, \
             tc.tile_pool(name="pp", bufs=1, space="PSUM") as ppool:
            t = _alloc_common(nc, pool, ppool)
            u_sb = pool.tile([128, 3 * L], F32)
            u3 = useg.rearrange("c (p t) -> c p t", p=128)
            for i in range(3):
                nc.gpsimd.dma_start(u_sb[:, i * L:(i + 1) * L], u3[i])
            nc.gpsimd.dma_start(t.b[:, :], bseed[:, :])
            nc.gpsimd.dma_start(t.at[:, :], atin[:, :])
            # c_i = B[i,0]*u0 + B[i,1]*u1 + B[i,2]*u2 + bA_i
            for i in range(3):
                ci = t.c[:, i * L:(i + 1) * L]
                nc.vector.tensor_scalar(
                    ci, u_sb[:, 0:L], Bm[i][0], bAv[i],
                    op0=AOP.mult, op1=AOP.add)
                nc.vector.scalar_tensor_tensor(
                    ci, u_sb[:, L:2 * L], Bm[i][1], ci,
                    op0=AOP.mult, op1=AOP.add)
                nc.vector.scalar_tensor_tensor(
                    ci, u_sb[:, 2 * L:3 * L], Bm[i][2], ci,
                    op0=AOP.mult, op1=AOP.add)
            # y init: y[c, :] = b_c
            nc.tensor.transpose(t.pb[:, :], t.b[:, 0:CPC], t.ident[0:3, 0:3])
            nc.vector.tensor_copy(t.bcols[:, :], t.pb[:, :])
            for i in range(3):
                nc.vector.tensor_scalar(
                    t.y[:, i * L:(i + 1) * L],
                    t.zeros[:, i * L:(i + 1) * L],
                    t.bcols[:, i:i + 1], None, op0=AOP.add)
            for k in k_list:
                _emit_sweep(nc, t, k, dtA, dtf)
            nc.sync.dma_start(y_out[:, :], t.y[:, :])
            nc.sync.dma_start(c_out[:, :], t.c[:, :])
            nc.sync.dma_start(sm_out[:, :], t.small[:, :])
            nc.sync.dma_start(b_out[:, :], t.b[:, :])
    nc.finalize()
    return nc


# --------------------------------------------------------------------------
# launch B
# --------------------------------------------------------------------------
def build_B(dtA, dtf, k_list):
    nc = bacc.Bacc("TRN2")
    y_in = nc.dram_tensor("y_in", [128, 3 * L], F32, kind="ExternalInput")
    c_in = nc.dram_tensor("c_in", [128, 3 * L], F32, kind="ExternalInput")
    b_in = nc.dram_tensor("b_in", [3, CPC + 4], F32, kind="ExternalInput")
    epsin = nc.dram_tensor("epsin", [3, 4], F32, kind="ExternalInput")
    atin = nc.dram_tensor("atin", [3, 4], F32, kind="ExternalInput")
    w1t = nc.dram_tensor("w1t", [6, 256], BF16, kind="ExternalInput")
    b1r = nc.dram_tensor("b1r", [2, 128], F32, kind="ExternalInput")
    w2t = nc.dram_tensor("w2t", [128, 16], F16, kind="ExternalInput")
    b2c = nc.dram_tensor("b2c", [3, 4], F32, kind="ExternalInput")
    memout = nc.dram_tensor("memout", [3, SEG], F32, kind="ExternalOutput")
    outdram = nc.dram_tensor("outdram", [3, SEG], F32, kind="ExternalOutput")

    with TileContext(nc) as tc:
        with tc.tile_pool(name="p", bufs=1) as pool:
          with tc.tile_pool(name="pp", bufs=1, space="PSUM") as ppool:
            t = _alloc_common(nc, pool, ppool)
            nc.gpsimd.dma_start(t.y[:, :], y_in[:, :])
            nc.gpsimd.dma_start(t.c[:, :], c_in[:, :])
            nc.gpsimd.dma_start(t.b[:, :], b_in[:, :])
            nc.gpsimd.dma_start(t.at[:, :], atin[:, :])
            eps_sb = pool.tile([3, 4], F32)
            nc.gpsimd.dma_start(eps_sb[:, :], epsin[:, :])
            w1_sb = pool.tile([6, 256], BF16)
            nc.gpsimd.dma_start(w1_sb[:, :], w1t[:, :])
            b1lo = pool.tile([128, 2], F32)
            nc.gpsimd.dma_start(b1lo[:, 0:1],
                              b1r[0:1, :].rearrange("o p -> p o"))
            nc.gpsimd.dma_start(b1lo[:, 1:2],
                              b1r[1:2, :].rearrange("o p -> p o"))
            w2_sb = pool.tile([128, 16], F16)
            nc.gpsimd.dma_start(w2_sb[:, :], w2t[:, :])
            b2_sb = pool.tile([3, 4], F32)
            nc.gpsimd.dma_start(b2_sb[:, :], b2c[:, :])
            mem = pool.tile([128, 3 * L], F32)
            xh = pool.tile([128, 3 * L], BF16)
            xl = pool.tile([128, 3 * L], BF16)

            first = True
            for k in k_list:
                _emit_sweep(nc, t, k, dtA, dtf,
                            eps_ap=(eps_sb[:, 0:1] if first else None))
                first = False
            # membrane: mem[c,tau] = (b_c + incl[c,tau]) * dt   fp32
            for i in range(3):
                nc.vector.tensor_scalar(
                    mem[:, i * L:(i + 1) * L], t.incl[:, i * L:(i + 1) * L],
                    t.bcols[:, i:i + 1], dtf, op0=AOP.add, op1=AOP.mult)
            m3 = memout.rearrange("c (p t) -> c p t", p=128)
            for i in range(3):
                nc.sync.dma_start(m3[i], mem[:, i * L:(i + 1) * L])
            # bf16 hi/lo split of the membrane for full-precision bf16 matmuls
            nc.vector.tensor_copy(xh[:, :], mem[:, :])
            nc.vector.tensor_tensor(xl[:, :], mem[:, :], xh[:, :], AOP.subtract)

          # ---------------- MLP ----------------
          if True:
            with tc.tile_pool(name="mx", bufs=2) as mx, \
                 tc.tile_pool(name="mh", bufs=2) as mh, \
                 tc.tile_pool(name="mph", bufs=2, space="PSUM") as mph, \
                 tc.tile_pool(name="mpo", bufs=2, space="PSUM") as mpo:
                for nb in range(NBLK):
                    xm = mx.tile([6, BLK], BF16, tag="xm")
                    for i in range(3):
                        nc.sync.dma_start(
                            xm[i:i + 1, :],
                            xh[nb * CHB:(nb + 1) * CHB, i * L:(i + 1) * L])
                        nc.sync.dma_start(
                            xm[3 + i:4 + i, :],
                            xl[nb * CHB:(nb + 1) * CHB, i * L:(i + 1) * L])
                    for half in range(4):
                        pso = mpo.tile([3, BLK // 4], F32, tag="pso")
                        for nt in range(2):
                            ns = half * 2 + nt
                            xsl = xm[:, ns * 512:(ns + 1) * 512]
                            psh = mph.tile([128, 512], F32, tag="psh")
                            psh2 = mph.tile([128, 512], F32, tag="psh2")
                            nc.tensor.matmul(psh[:, :], w1_sb[:, 0:128], xsl,
                                             start=True, stop=True)
                            nc.tensor.matmul(psh2[:, :], w1_sb[:, 128:256], xsl,
                                             start=True, stop=True)
                            hlo = mh.tile([128, 512], F16, tag="hlo")
                            hhi = mh.tile([128, 512], F16, tag="hhi")
                            nc.scalar.activation(hlo[:, :], psh[:, :], AFT.Relu,
                                                 bias=b1lo[:, 0:1], scale=1.0)
                            nc.vector.tensor_scalar(
                                hhi[:, :], psh2[:, :], b1lo[:, 1:2], 0.0,
                                op0=AOP.add, op1=AOP.max)
                            osl = pso[:, nt * 512:(nt + 1) * 512]
                            nc.tensor.matmul(osl, w2_sb[:, 0:3], hlo[:, :],
                                             start=True, stop=False)
                            nc.tensor.matmul(osl, w2_sb[:, 3:6], hhi[:, :],
                                             start=False, stop=False)
                            nc.tensor.matmul(osl, w2_sb[:, 8:11], hlo[:, :],
                                             start=False, stop=False)
                            nc.tensor.matmul(osl, w2_sb[:, 11:14], hhi[:, :],
                                             start=False, stop=True)
                        osb = mh.tile([3, BLK // 4], F32, tag="osb")
                        if half % 2 == 0:
                            nc.scalar.activation(osb[:, :], pso[:, :], AFT.Identity,
                                                 bias=b2_sb[:, 0:1], scale=1.0)
                        else:
                            nc.vector.tensor_scalar(
                                osb[:, :], pso[:, :], b2_sb[:, 0:1], None,
                                op0=AOP.add)
                        nc.sync.dma_start(
                            outdram[:, nb * BLK + half * (BLK // 4):
                                    nb * BLK + (half + 1) * (BLK // 4)],
                            osb[:, :])
    nc.finalize()
    return nc


# --------------------------------------------------------------------------
# host orchestration
# --------------------------------------------------------------------------
def _coarse_seed(A64, c64, dt):
    """Frozen-x coarse boundary seed, fp64, in y = x/dt units. [C+1, 3]"""
    C = S // L
    cc = c64.reshape(3, C, L)
    b = np.zeros((C + 1, 3))
    for c in range(C):
        z = (dt * (A64 @ b[c]))[:, None] + cc[:, c, :]
        b[c + 1] = b[c] + np.tanh(z).sum(axis=1)
    return b


def kernel(u, dt, A, B, bA, W1, b1, W2, b2):
    u = np.ascontiguousarray(np.asarray(u, dtype=np.float32))
    dtf = float(np.float32(np.asarray(dt)))
    A32 = np.asarray(A, dtype=np.float32)
    B32 = np.asarray(B, dtype=np.float32)
    bA32 = np.asarray(bA, dtype=np.float32)
    W1 = np.asarray(W1, dtype=np.float32)
    b1 = np.asarray(b1, dtype=np.float32)
    W2 = np.asarray(W2, dtype=np.float32)
    b2 = np.asarray(b2, dtype=np.float32)

    dtA = [[float(np.float32(dtf * A32[i, k])) for k in range(3)]
           for i in range(3)]
    Bm = [[float(B32[i, k]) for k in range(3)] for i in range(3)]
    bAv = [float(bA32[i]) for i in range(3)]

    key = (dtf, tuple(map(tuple, dtA)))
    if key not in _CACHE:
        _CACHE[key] = (build_A(dtA, Bm, bAv, dtf, [6, 3]),
                       build_B(dtA, dtf, [6, 3]))
    ncA, ncB = _CACHE[key]

    # host-side coarse seed (y units)
    A64 = A32.astype(np.float64)
    c64 = B32.astype(np.float64) @ u.astype(np.float64) \
        + bA32.astype(np.float64)[:, None]
    b_seed = _coarse_seed(A64, c64, dtf)            # [C+1, 3], y units
    atmat = np.zeros((3, 4), np.float32)
    atmat[:, 0:3] = A32.T

    in_A = []
    for k in range(NCORE):
        bs = np.zeros((3, CPC + 4), np.float32)
        bs[:, 0:CPC + 1] = b_seed[k * CPC:(k + 1) * CPC + 1].T
        in_A.append({
            "useg": np.ascontiguousarray(u[:, k * SEG:(k + 1) * SEG]),
            "bseed": bs,
            "atin": atmat,
        })
    resA = run_bass_kernel_spmd(ncA, in_A, core_ids=list(range(NCORE)))
    ra = resA.results
    LAST_TIMES["A"] = resA.exec_time_ns

    # host: compose per-core first-order boundary maps -> eps per core
    eps = np.zeros((NCORE, 3), np.float64)
    e_prev = np.zeros(3)
    for k in range(NCORE):
        eps[k] = e_prev
        sth = ra[k]["sm_out"][:, 3:6].astype(np.float64)      # [128,3]
        SD = dtf * (L - sth)
        Phi = np.eye(3)
        for c in range(CPC):
            Phi = (np.eye(3) + np.diag(SD[c]) @ A64) @ Phi
        E_k = ra[k]["b_out"][:, CPC].astype(np.float64)
        e_prev = (E_k + Phi @ e_prev) - b_seed[(k + 1) * CPC]

    # weights packing for launch B
    import ml_dtypes
    w1t = np.zeros((6, 256), ml_dtypes.bfloat16)
    w1t[0:3, :] = W1.T.astype(ml_dtypes.bfloat16)
    w1t[3:6, :] = w1t[0:3, :]
    b1r = np.ascontiguousarray(b1.reshape(2, 128))
    w2t = np.zeros((128, 16), np.float16)
    w2hi_a = W2[:, 0:128].T.astype(np.float16)
    w2hi_b = W2[:, 128:256].T.astype(np.float16)
    w2t[:, 0:3] = w2hi_a
    w2t[:, 3:6] = w2hi_b
    w2t[:, 8:11] = (W2[:, 0:128].T - w2hi_a.astype(np.float32)).astype(np.float16)
    w2t[:, 11:14] = (W2[:, 128:256].T - w2hi_b.astype(np.float32)).astype(np.float16)
    b2c = np.zeros((3, 4), np.float32)
    b2c[:, 0] = b2

    in_B = []
    for k in range(NCORE):
        ep = np.zeros((3, 4), np.float32)
        ep[:, 0] = eps[k].astype(np.float32)
        in_B.append({
            "y_in": ra[k]["y_out"], "c_in": ra[k]["c_out"],
            "b_in": ra[k]["b_out"], "epsin": ep, "atin": atmat,
            "w1t": w1t, "b1r": b1r, "w2t": w2t, "b2c": b2c,
        })
    resB = run_bass_kernel_spmd(ncB, in_B, core_ids=list(range(NCORE)))
    rb = resB.results
    LAST_TIMES["B"] = resB.exec_time_ns

    outputs = np.concatenate([rb[k]["outdram"] for k in range(NCORE)], axis=1)
    membrane = np.concatenate([rb[k]["memout"] for k in range(NCORE)], axis=1)
    return outputs.astype(np.float32), membrane.astype(np.float32)


# revision 17
# speedup vs baseline: 1.1467x; 1.1467x over previous
"""Trainium2 Bass kernel for the RNODE (ODE-RNN) + per-step MLP model.

Model (reference):
    x_{t+1} = x_t + dt*tanh(A x_t + B u_t + bA)       (sequential, S=262144 steps)
    mem[:, t]  = x_{t+1}
    out = W2 @ relu(W1 @ mem + b1) + b2
    returns (out, mem), both [3, S] fp32.

Parallel-in-time strategy (8 cores, data-parallel over the time axis):
  Work in scaled units y = x/dt so the recurrence is y_{t+1} = y_t + tanh(z_t),
  z_t = (dt*A) y_t + c_t, c = B u + bA.  Each core owns a 32768-step segment
  laid out as 128 chunks (partitions) x 256 steps (free dim).

  Newton/Picard iteration per core ("sweep"):
    1. z = dtA@y + c (fused scalar_tensor_tensor chains), th = tanh(z) (ACT)
    2. per-chunk inclusive prefix sums of th via tensor_tensor_scan
    3. chunk-boundary correction: solve the linearized boundary recurrence
       delta_{c+1} = (I + diag(SD_c) A) delta_c + r_c across the 128 chunks
       (SD_c = dt * sum sech^2, first-order chunk Jacobian) by inner Picard
       iterations using a [3,128]-layout scan + a tiny PE matmul for A@delta.
    4. rebuild trajectory y from corrected boundaries + prefix sums.

  Cross-core coupling is resolved with two launches: launch A runs sweeps from
  a cheap coarse seed; the host composes the 8 per-core first-order boundary
  maps (8 tiny 3x3 affine maps) into per-core incoming-state corrections eps;
  launch B applies eps (delta-scan initial value), re-sweeps, then evaluates
  the MLP on the tensor engine (W1/W2 matmuls, relu+bias fused in the
  PSUM->SBUF copies on ACT/DVE) and streams both outputs to DRAM.

The dynamics saturate tanh (|x| grows to ~84), so sech^2 ~ 0 over most of the
sequence and the first-order chunk Jacobians are nearly exact; the iteration
converges to well inside the fp32 envelope of the sequential reference
(validated against an fp64 sequential solve).
"""

import numpy as np

import concourse.bass as bass
import concourse.bacc as bacc
import concourse.mybir as mybir
from concourse.bass_utils import run_bass_kernel_spmd
from concourse.masks import make_identity
from concourse.tile import TileContext

F32 = mybir.dt.float32
BF16 = mybir.dt.bfloat16
F16 = mybir.dt.float16
AOP = mybir.AluOpType
AFT = mybir.ActivationFunctionType

S = 262144
NCORE = 8
SEG = S // NCORE          # 32768 steps per core
L = 256                   # chunk length (free dim)
CPC = SEG // L            # 128 chunks per core (partition dim)
NBLK = 8                  # MLP time blocks per core
BLK = SEG // NBLK         # 4096 steps per MLP block
CHB = BLK // L            # 16 chunks per MLP block

_CACHE = {}
LAST_TIMES = {}   # filled with exec_time_ns per launch when tracing is on


# --------------------------------------------------------------------------
# shared sweep emitter
# --------------------------------------------------------------------------
class T:  # tile namespace
    pass


def _alloc_common(nc, pool, ppool):
    t = T()
    t.ident = pool.tile([128, 128], F32)
    make_identity(nc, t.ident[:, :])
    t.zeros = pool.tile([128, 3 * L], F32)
    nc.vector.memset(t.zeros[:, :], 0.0)
    t.zeros3 = pool.tile([3, CPC + 4], F32)
    nc.vector.memset(t.zeros3[:, :], 0.0)
    t.ones3 = pool.tile([3, CPC + 4], F32)
    nc.vector.memset(t.ones3[:, :], 1.0)
    t.c = pool.tile([128, 3 * L], F32)
    t.y = pool.tile([128, 3 * L], F32)
    t.z = pool.tile([128, 3 * L], F32)
    t.th = pool.tile([128, 3 * L], F32)
    t.incl = pool.tile([128, 3 * L], F32)
    t.junk = pool.tile([128, L], F32)
    t.small = pool.tile([128, 8], F32)
    t.b = pool.tile([3, CPC + 4], F32)
    t.delta = pool.tile([3, CPC + 4], F32)
    t.r = pool.tile([3, CPC], F32)
    t.g = pool.tile([3, CPC], F32)
    t.SDp = pool.tile([3, CPC], F32)
    t.bcols = pool.tile([128, 3], F32)
    t.at = pool.tile([3, 4], F32)
    t.ptT = ppool.tile([3, 128], F32)
    t.ptS = ppool.tile([3, 128], F32)
    t.pb = ppool.tile([128, 3], F32)
    t.psA = ppool.tile([3, CPC], F32)
    return t


def _emit_sweep(nc, t, k_in, dtA, dtf, eps_ap=None):
    """One outer iteration: z/tanh/prefix + chunk-boundary solve + rebuild."""
    # z_i = sum_k dtA[i,k]*y_k + c_i
    for i in range(3):
        zi = t.z[:, i * L:(i + 1) * L]
        nc.vector.scalar_tensor_tensor(
            zi, t.y[:, 0:L], dtA[i][0], t.c[:, i * L:(i + 1) * L],
            op0=AOP.mult, op1=AOP.add)
        nc.vector.scalar_tensor_tensor(
            zi, t.y[:, L:2 * L], dtA[i][1], zi, op0=AOP.mult, op1=AOP.add)
        nc.vector.scalar_tensor_tensor(
            zi, t.y[:, 2 * L:3 * L], dtA[i][2], zi, op0=AOP.mult, op1=AOP.add)
    nc.scalar.activation(t.th[:, :], t.z[:, :], AFT.Tanh)
    for i in range(3):
        thi = t.th[:, i * L:(i + 1) * L]
        # junk = th^2, accum_out = sum(th^2) per chunk  (for SD)
        nc.vector.scalar_tensor_tensor(
            t.junk[:, :], thi, 1.0, thi, op0=AOP.mult, op1=AOP.mult,
            accum_out=t.small[:, 3 + i:4 + i])
        # inclusive prefix sum of th along the chunk
        nc.vector.tensor_tensor_scan(
            t.incl[:, i * L:(i + 1) * L], t.zeros[:, 0:L], thi, 0.0,
            op0=AOP.add, op1=AOP.add)
        nc.vector.tensor_copy(
            t.small[:, i:i + 1], t.incl[:, (i + 1) * L - 1:(i + 1) * L])
    # chunk summaries -> [*,128] layout
    nc.tensor.transpose(t.ptT[:, :], t.small[:, 0:3], t.ident[:, :])
    nc.tensor.transpose(t.ptS[:, :], t.small[:, 3:6], t.ident[:, :])
    # SD' = dt*(L - sum th^2) = (sth * -dt) + L*dt
    nc.vector.tensor_scalar(
        t.SDp[:, :], t.ptS[:, :], -dtf, float(L) * dtf,
        op0=AOP.mult, op1=AOP.add)
    # defect r_c = b_c + T_c - b_{c+1}
    nc.vector.tensor_tensor(t.r[:, :], t.b[:, 0:CPC], t.ptT[:, :], AOP.add)
    nc.vector.tensor_tensor(t.r[:, :], t.r[:, :], t.b[:, 1:CPC + 1], AOP.subtract)
    # delta init
    if eps_ap is not None:
        nc.vector.tensor_scalar(
            t.delta[:, 0:CPC + 1], t.ones3[:, 0:CPC + 1], eps_ap, None,
            op0=AOP.mult)
    else:
        nc.vector.memset(t.delta[:, 0:CPC + 1], 0.0)
    # inner Picard on the boundary recurrence
    for _ in range(k_in):
        nc.tensor.matmul(t.psA[:, :], t.at[0:3, 0:3], t.delta[:, 0:CPC],
                         start=True, stop=True)
        nc.vector.tensor_tensor(t.g[:, :], t.SDp[:, :], t.psA[:, :], AOP.mult)
        nc.vector.tensor_tensor(t.g[:, :], t.g[:, :], t.r[:, :], AOP.add)
        nc.vector.tensor_tensor_scan(
            t.delta[:, 1:CPC + 1], t.zeros3[:, 0:CPC], t.g[:, :],
            t.delta[:, 0:1], op0=AOP.add, op1=AOP.add)
    nc.vector.tensor_tensor(t.b[:, 0:CPC + 1], t.b[:, 0:CPC + 1],
                            t.delta[:, 0:CPC + 1], AOP.add)
    # refresh per-chunk boundary columns and rebuild y
    nc.tensor.transpose(t.pb[:, :], t.b[:, 0:CPC], t.ident[0:3, 0:3])
    nc.vector.tensor_copy(t.bcols[:, :], t.pb[:, :])
    for i in range(3):
        nc.vector.tensor_copy(t.y[:, i * L:i * L + 1], t.bcols[:, i:i + 1])
        nc.vector.tensor_scalar(
            t.y[:, i * L + 1:(i + 1) * L], t.incl[:, i * L:(i + 1) * L - 1],
            t.bcols[:, i:i + 1], None, op0=AOP.add)


# --------------------------------------------------------------------------
# launch A
# --------------------------------------------------------------------------
def build_A(dtA, Bm, bAv, dtf, k_list):
    nc = bacc.Bacc("TRN2")
    useg = nc.dram_tensor("useg", [3, SEG], F32, kind="ExternalInput")
    bseed = nc.dram_tensor("bseed", [3, CPC + 4], F32, kind="ExternalInput")
    atin = nc.dram_tensor("atin", [3, 4], F32, kind="ExternalInput")
    y_out = nc.dram_tensor("y_out", [128, 3 * L], F32, kind="ExternalOutput")
    c_out = nc.dram_tensor("c_out", [128, 3 * L], F32, kind="ExternalOutput")
    sm_out = nc.dram_tensor("sm_out", [128, 8], F32, kind="ExternalOutput")
    b_out = nc.dram_tensor("b_out", [3, CPC + 4], F32, kind="ExternalOutput")

    with TileContext(nc) as tc:
        with tc.tile_pool(name="p", bufs=1) as pool, \
             tc.tile_pool(name="pp", bufs=1, space="PSUM") as ppool:
            t = _alloc_common(nc, pool, ppool)
            u_sb = pool.tile([128, 3 * L], F32)
            u3 = useg.rearrange("c (p t) -> c p t", p=128)
            for i in range(3):
                nc.gpsimd.dma_start(u_sb[:, i * L:(i + 1) * L], u3[i])
            nc.gpsimd.dma_start(t.b[:, :], bseed[:, :])
            nc.gpsimd.dma_start(t.at[:, :], atin[:, :])
            # c_i = B[i,0]*u0 + B[i,1]*u1 + B[i,2]*u2 + bA_i
            for i in range(3):
                ci = t.c[:, i * L:(i + 1) * L]
                nc.vector.tensor_scalar(
                    ci, u_sb[:, 0:L], Bm[i][0], bAv[i],
                    op0=AOP.mult, op1=AOP.add)
                nc.vector.scalar_tensor_tensor(
                    ci, u_sb[:, L:2 * L], Bm[i][1], ci,
                    op0=AOP.mult, op1=AOP.add)
                nc.vector.scalar_tensor_tensor(
                    ci, u_sb[:, 2 * L:3 * L], Bm[i][2], ci,
                    op0=AOP.mult, op1=AOP.add)
            # y init: y[c, :] = b_c
            nc.tensor.transpose(t.pb[:, :], t.b[:, 0:CPC], t.ident[0:3, 0:3])
            nc.vector.tensor_copy(t.bcols[:, :], t.pb[:, :])
            for i in range(3):
                nc.vector.tensor_scalar(
                    t.y[:, i * L:(i + 1) * L],
                    t.zeros[:, i * L:(i + 1) * L],
                    t.bcols[:, i:i + 1], None, op0=AOP.add)
            for k in k_list:
                _emit_sweep(nc, t, k, dtA, dtf)
            nc.sync.dma_start(y_out[:, :], t.y[:, :])
            nc.sync.dma_start(c_out[:, :], t.c[:, :])
            nc.sync.dma_start(sm_out[:, :], t.small[:, :])
            nc.sync.dma_start(b_out[:, :], t.b[:, :])
    nc.finalize()
    return nc


# --------------------------------------------------------------------------
# launch B
# --------------------------------------------------------------------------
def build_B(dtA, dtf, k_list):
    nc = bacc.Bacc("TRN2")
    y_in = nc.dram_tensor("y_in", [128, 3 * L], F32, kind="ExternalInput")
    c_in = nc.dram_tensor("c_in", [128, 3 * L], F32, kind="ExternalInput")
    b_in = nc.dram_tensor("b_in", [3, CPC + 4], F32, kind="ExternalInput")
    epsin = nc.dram_tensor("epsin", [3, 4], F32, kind="ExternalInput")
    atin = nc.dram_tensor("atin", [3, 4], F32, kind="ExternalInput")
    w1t = nc.dram_tensor("w1t", [6, 256], BF16, kind="ExternalInput")
    b1r = nc.dram_tensor("b1r", [2, 128], F32, kind="ExternalInput")
    w2t = nc.dram_tensor("w2t", [128, 8], F16, kind="ExternalInput")
    b2c = nc.dram_tensor("b2c", [3, 4], F32, kind="ExternalInput")
    memout = nc.dram_tensor("memout", [3, SEG], F32, kind="ExternalOutput")
    outdram = nc.dram_tensor("outdram", [3, SEG], F32, kind="ExternalOutput")

    with TileContext(nc) as tc:
        with tc.tile_pool(name="p", bufs=1) as pool:
          with tc.tile_pool(name="pp", bufs=1, space="PSUM") as ppool:
            t = _alloc_common(nc, pool, ppool)
            nc.gpsimd.dma_start(t.y[:, :], y_in[:, :])
            nc.gpsimd.dma_start(t.c[:, :], c_in[:, :])
            nc.gpsimd.dma_start(t.b[:, :], b_in[:, :])
            nc.gpsimd.dma_start(t.at[:, :], atin[:, :])
            eps_sb = pool.tile([3, 4], F32)
            nc.gpsimd.dma_start(eps_sb[:, :], epsin[:, :])
            w1_sb = pool.tile([6, 256], BF16)
            nc.gpsimd.dma_start(w1_sb[:, :], w1t[:, :])
            b1lo = pool.tile([128, 2], F32)
            nc.gpsimd.dma_start(b1lo[:, 0:1],
                              b1r[0:1, :].rearrange("o p -> p o"))
            nc.gpsimd.dma_start(b1lo[:, 1:2],
                              b1r[1:2, :].rearrange("o p -> p o"))
            w2_sb = pool.tile([128, 8], F16)
            nc.gpsimd.dma_start(w2_sb[:, :], w2t[:, :])
            b2_sb = pool.tile([3, 4], F32)
            nc.gpsimd.dma_start(b2_sb[:, :], b2c[:, :])
            mem = pool.tile([128, 3 * L], F32)
            xh = pool.tile([128, 3 * L], BF16)
            xl = pool.tile([128, 3 * L], BF16)

            first = True
            for k in k_list:
                _emit_sweep(nc, t, k, dtA, dtf,
                            eps_ap=(eps_sb[:, 0:1] if first else None))
                first = False
            # membrane: mem[c,tau] = (b_c + incl[c,tau]) * dt   fp32
            for i in range(3):
                nc.vector.tensor_scalar(
                    mem[:, i * L:(i + 1) * L], t.incl[:, i * L:(i + 1) * L],
                    t.bcols[:, i:i + 1], dtf, op0=AOP.add, op1=AOP.mult)
            m3 = memout.rearrange("c (p t) -> c p t", p=128)
            for i in range(3):
                nc.sync.dma_start(m3[i], mem[:, i * L:(i + 1) * L])
            # bf16 hi/lo split of the membrane for full-precision bf16 matmuls
            nc.vector.tensor_copy(xh[:, :], mem[:, :])
            nc.vector.tensor_tensor(xl[:, :], mem[:, :], xh[:, :], AOP.subtract)

          # ---------------- MLP ----------------
          if True:
            with tc.tile_pool(name="mx", bufs=2) as mx, \
                 tc.tile_pool(name="mh", bufs=2) as mh, \
                 tc.tile_pool(name="mph", bufs=2, space="PSUM") as mph, \
                 tc.tile_pool(name="mpo", bufs=2, space="PSUM") as mpo:
                for nb in range(NBLK):
                    xm = mx.tile([6, BLK], BF16, tag="xm")
                    for i in range(3):
                        nc.sync.dma_start(
                            xm[i:i + 1, :],
                            xh[nb * CHB:(nb + 1) * CHB, i * L:(i + 1) * L])
                        nc.sync.dma_start(
                            xm[3 + i:4 + i, :],
                            xl[nb * CHB:(nb + 1) * CHB, i * L:(i + 1) * L])
                    for half in range(4):
                        pso = mpo.tile([3, BLK // 4], F32, tag="pso")
                        for nt in range(2):
                            ns = half * 2 + nt
                            xsl = xm[:, ns * 512:(ns + 1) * 512]
                            psh = mph.tile([128, 512], F32, tag="psh")
                            psh2 = mph.tile([128, 512], F32, tag="psh2")
                            nc.tensor.matmul(psh[:, :], w1_sb[:, 0:128], xsl,
                                             start=True, stop=True)
                            nc.tensor.matmul(psh2[:, :], w1_sb[:, 128:256], xsl,
                                             start=True, stop=True)
                            hlo = mh.tile([128, 512], F16, tag="hlo")
                            hhi = mh.tile([128, 512], F16, tag="hhi")
                            nc.scalar.activation(hlo[:, :], psh[:, :], AFT.Relu,
                                                 bias=b1lo[:, 0:1], scale=1.0)
                            nc.vector.tensor_scalar(
                                hhi[:, :], psh2[:, :], b1lo[:, 1:2], 0.0,
                                op0=AOP.add, op1=AOP.max)
                            osl = pso[:, nt * 512:(nt + 1) * 512]
                            nc.tensor.matmul(osl, w2_sb[:, 0:3], hlo[:, :],
                                             start=True, stop=False)
                            nc.tensor.matmul(osl, w2_sb[:, 3:6], hhi[:, :],
                                             start=False, stop=True)
                        osb = mh.tile([3, BLK // 4], F32, tag="osb")
                        if half % 2 == 0:
                            nc.scalar.activation(osb[:, :], pso[:, :], AFT.Identity,
                                                 bias=b2_sb[:, 0:1], scale=1.0)
                        else:
                            nc.vector.tensor_scalar(
                                osb[:, :], pso[:, :], b2_sb[:, 0:1], None,
                                op0=AOP.add)
                        nc.sync.dma_start(
                            outdram[:, nb * BLK + half * (BLK // 4):
                                    nb * BLK + (half + 1) * (BLK // 4)],
                            osb[:, :])
    nc.finalize()
    return nc


# --------------------------------------------------------------------------
# host orchestration
# --------------------------------------------------------------------------
def _coarse_seed(A64, c64, dt):
    """Frozen-x coarse boundary seed, fp64, in y = x/dt units. [C+1, 3]"""
    C = S // L
    cc = c64.reshape(3, C, L)
    b = np.zeros((C + 1, 3))
    for c in range(C):
        z = (dt * (A64 @ b[c]))[:, None] + cc[:, c, :]
        b[c + 1] = b[c] + np.tanh(z).sum(axis=1)
    return b


def kernel(u, dt, A, B, bA, W1, b1, W2, b2):
    u = np.ascontiguousarray(np.asarray(u, dtype=np.float32))
    dtf = float(np.float32(np.asarray(dt)))
    A32 = np.asarray(A, dtype=np.float32)
    B32 = np.asarray(B, dtype=np.float32)
    bA32 = np.asarray(bA, dtype=np.float32)
    W1 = np.asarray(W1, dtype=np.float32)
    b1 = np.asarray(b1, dtype=np.float32)
    W2 = np.asarray(W2, dtype=np.float32)
    b2 = np.asarray(b2, dtype=np.float32)

    dtA = [[float(np.float32(dtf * A32[i, k])) for k in range(3)]
           for i in range(3)]
    Bm = [[float(B32[i, k]) for k in range(3)] for i in range(3)]
    bAv = [float(bA32[i]) for i in range(3)]

    key = (dtf, tuple(map(tuple, dtA)))
    if key not in _CACHE:
        _CACHE[key] = (build_A(dtA, Bm, bAv, dtf, [6, 3]),
                       build_B(dtA, dtf, [6, 3]))
    ncA, ncB = _CACHE[key]

    # host-side coarse seed (y units)
    A64 = A32.astype(np.float64)
    c64 = B32.astype(np.float64) @ u.astype(np.float64) \
        + bA32.astype(np.float64)[:, None]
    b_seed = _coarse_seed(A64, c64, dtf)            # [C+1, 3], y units
    atmat = np.zeros((3, 4), np.float32)
    atmat[:, 0:3] = A32.T

    in_A = []
    for k in range(NCORE):
        bs = np.zeros((3, CPC + 4), np.float32)
        bs[:, 0:CPC + 1] = b_seed[k * CPC:(k + 1) * CPC + 1].T
        in_A.append({
            "useg": np.ascontiguousarray(u[:, k * SEG:(k + 1) * SEG]),
            "bseed": bs,
            "atin": atmat,
        })
    resA = run_bass_kernel_spmd(ncA, in_A, core_ids=list(range(NCORE)))
    ra = resA.results
    LAST_TIMES["A"] = resA.exec_time_ns

    # host: compose per-core first-order boundary maps -> eps per core
    eps = np.zeros((NCORE, 3), np.float64)
    e_prev = np.zeros(3)
    for k in range(NCORE):
        eps[k] = e_prev
        sth = ra[k]["sm_out"][:, 3:6].astype(np.float64)      # [128,3]
        SD = dtf * (L - sth)
        Phi = np.eye(3)
        for c in range(CPC):
            Phi = (np.eye(3) + np.diag(SD[c]) @ A64) @ Phi
        E_k = ra[k]["b_out"][:, CPC].astype(np.float64)
        e_prev = (E_k + Phi @ e_prev) - b_seed[(k + 1) * CPC]

    # weights packing for launch B
    import ml_dtypes
    w1t = np.zeros((6, 256), ml_dtypes.bfloat16)
    w1t[0:3, :] = W1.T.astype(ml_dtypes.bfloat16)
    w1t[3:6, :] = w1t[0:3, :]
    b1r = np.ascontiguousarray(b1.reshape(2, 128))
    w2t = np.zeros((128, 8), np.float16)
    w2t[:, 0:3] = W2[:, 0:128].T.astype(np.float16)
    w2t[:, 3:6] = W2[:, 128:256].T.astype(np.float16)
    b2c = np.zeros((3, 4), np.float32)
    b2c[:, 0] = b2

    in_B = []
    for k in range(NCORE):
        ep = np.zeros((3, 4), np.float32)
        ep[:, 0] = eps[k].astype(np.float32)
        in_B.append({
            "y_in": ra[k]["y_out"], "c_in": ra[k]["c_out"],
            "b_in": ra[k]["b_out"], "epsin": ep, "atin": atmat,
            "w1t": w1t, "b1r": b1r, "w2t": w2t, "b2c": b2c,
        })
    resB = run_bass_kernel_spmd(ncB, in_B, core_ids=list(range(NCORE)))
    rb = resB.results
    LAST_TIMES["B"] = resB.exec_time_ns

    outputs = np.concatenate([rb[k]["outdram"] for k in range(NCORE)], axis=1)
    membrane = np.concatenate([rb[k]["memout"] for k in range(NCORE)], axis=1)
    return outputs.astype(np.float32), membrane.astype(np.float32)
